# revision 54
# baseline (speedup 1.0000x reference)
"""EntityAttention Trainium2 kernel (nn_EntityAttention_31525059952740).

Math (per (batch, entity) group n, all 64 events e):
  q = (events @ Wq.T + bq) * scale            shared across n     [64, 512]
  k = toks_b @ Wk.T + bk                      per batch           [512, 512]
  v = toks_b @ Wv.T + bv                      per batch           [512, 512]
  scores[h,e,s] = q_h[e] . k_h[s]             per batch (2 heads x 256)
  attn = softmax over s, masked by entities[n]  (mask = multiplicative
         0/1 on exp since exp(-1e9 + x) == 0 in fp32)
  out[e] = concat_h(attn_h @ v_h);  O = out @ Wo.T + bo

Sharding: batch b -> core b (8 batches, 8 cores). Each core computes all 16
entities of its batch -> output rows [1024, 512] per core, concatenated.

Device layout ("transposed attention"): scores^T [s(partitions), (h,e)] so
that the entity mask is a per-partition scalar (fused into a single DVE
tensor_scalar per (entity, s-chunk)) and softmax denominators come from one
tiny PE matmul  S = masks^T.T @ exp(scores^T).

All heavy matmuls run as float32r (full PE rate, ~1e-4 relative error).
"""

import numpy as np

import concourse.bass as bass
import concourse.tile as tile
import concourse.mybir as mybir
from concourse import bacc
from concourse.bass_utils import run_bass_kernel_spmd

NB, SL, NH, EN, NE, HEADS = 8, 512, 512, 16, 64, 2
DH = NH // HEADS          # 256
P = 128
NCHUNK = NH // P          # 4 chunks of the hidden dim
SCHUNK = SL // P          # 4 chunks of the sequence dim
SCALE = 1.0 / np.sqrt(DH).astype(np.float32)

F32 = mybir.dt.float32
F32R = mybir.dt.float32r

_CACHE = {}


def _build():
    nc = bacc.Bacc("TRN2", target_bir_lowering=False, debug=False, num_devices=NB)

    # ---- I/O ----
    toksT_d = nc.dram_tensor("toksT", [NH, SL], F32R, kind="ExternalInput").ap()
    wtil_d = nc.dram_tensor("wtil", [P, NCHUNK, HEADS * NE], F32R,
                            kind="ExternalInput").ap()
    # packed small inputs: [128, 64 masksT]
    smalls_d = nc.dram_tensor("smalls", [P, 64], F32R, kind="ExternalInput").ap()
    wvT_d = nc.dram_tensor("WvT", [NH, NH], F32R, kind="ExternalInput").ap()
    woT_d = nc.dram_tensor("WoT", [NH, NH], F32R, kind="ExternalInput").ap()
    out_d = nc.dram_tensor("out", [EN * NE, NH], F32, kind="ExternalOutput").ap()

    EXP = mybir.ActivationFunctionType.Exp
    CPY = mybir.ActivationFunctionType.Copy
    IDN = mybir.ActivationFunctionType.Identity

    with tile.TileContext(nc) as tc:
        with (
            tc.tile_pool(name="wpool", bufs=1) as wpool,
            tc.tile_pool(name="sb", bufs=1) as sb,
            tc.tile_pool(name="ostage", bufs=2) as ostage,
            tc.tile_pool(name="pbig", bufs=4, space="PSUM") as pbig,
            tc.tile_pool(name="psmall", bufs=4, space="PSUM") as psmall,
            tc.tile_pool(name="dram", bufs=1, space="DRAM") as dram,
        ):
            # ---------- loads ----------
            toksT_r = toksT_d.rearrange("(c p) s -> p c s", p=P)
            wtil_t = wpool.tile([P, NCHUNK, HEADS * NE], F32R, tag="wtil")
            nc.scalar.dma_start(wtil_t[:], wtil_d)
            wtil_sb = wtil_t[:]

            toksT_t = []
            for hc in range(NCHUNK):
                t = sb.tile([P, SL], F32R, tag=f"toksT{hc}")
                nc.sync.dma_start(t[:], toksT_r[:, hc, :])
                toksT_t.append(t)
            smalls = wpool.tile([P, 64], F32R, tag="smalls")
            nc.scalar.dma_start(smalls[:], smalls_d)
            masks_sb = smalls[:, 0:64].rearrange("p (c e) -> p c e", c=SCHUNK)

            wv_sb = wpool.tile([P, NCHUNK, NH], F32R, tag="wv")
            nc.sync.dma_start(wv_sb[:], wvT_d.rearrange("(c p) d -> p c d", p=P))

            def toksT(hc):
                return toksT_t[hc][:]

            wo_sb = wpool.tile([P, NCHUNK, NH], F32R, tag="wo")
            nc.sync.dma_start(wo_sb[:], woT_d.rearrange("(c p) d -> p c d", p=P))



            # ---------- scores^T = toksT.T @ Wtil -> E = exp  [s, (h,e)] -----
            # Wtil = (q_scaled @ Wk) folded on host; bk cancels in softmax.
            pss = [psmall.tile([P, HEADS * NE], F32, tag="psm", name=f"ps{sc}")
                   for sc in range(SCHUNK)]
            for sc in range(SCHUNK):
                for hc in range(NCHUNK):
                    nc.tensor.matmul(
                        pss[sc][:],
                        toksT(hc)[:, sc * P:(sc + 1) * P], wtil_sb[:, hc, :],
                        start=(hc == 0), stop=(hc == NCHUNK - 1),
                    )
            e_sbs = []
            for sc in range(SCHUNK):
                e_sb = sb.tile([P, HEADS * NE], F32R, tag=f"e{sc}")
                nc.scalar.activation(e_sb[:], pss[sc][:], EXP)
                e_sbs.append(e_sb)
            # ---------- S = masksT.T @ E ; recip; DRAM-roundtrip bcast -------
            pS = psmall.tile([EN, HEADS * NE], F32, tag="psm", name="pS")
            for sc in range(SCHUNK):
                nc.tensor.matmul(pS[:], masks_sb[:, sc, :], e_sbs[sc][:],
                                 start=(sc == 0), stop=(sc == SCHUNK - 1))
            srec = sb.tile([EN, HEADS * NE], F32, tag="srec")
            nc.vector.reciprocal(srec[:], pS[:])
            srec_dram = dram.tile([EN, HEADS * NE], F32)
            nc.sync.dma_start(srec_dram[:], srec[:])
            srec_bcs = []
            for grp in range(4):
                t = sb.tile([P, 4, HEADS * NE], F32, tag=f"srec_bc{grp}")
                sd_ap = srec_dram[grp * 4:(grp + 1) * 4, :]
                nc.sync.dma_start(
                    t[:],
                    bass.AP(tensor=sd_ap.tensor, offset=sd_ap.offset,
                            ap=[[0, P], *sd_ap.ap]),
                )
                srec_bcs.append(t)

            def srec_slice(grp, h):
                return srec_bcs[grp][:, :, h * NE:(h + 1) * NE]

            # ---------- V = toks @ WvT + bv (late PE filler) ----------
            vs = []
            for i in range(SCHUNK):
                pv = pbig.tile([P, NH], F32, tag="pb", name=f"pv{i}")
                for hc in range(NCHUNK):
                    nc.tensor.matmul(
                        pv[:], toksT(hc)[:, i * P:(i + 1) * P], wv_sb[:, hc, :],
                        start=(hc == 0), stop=(hc == NCHUNK - 1),
                    )
                v = sb.tile([P, NH], F32R, tag=f"v{i}")
                H = NH // 2
                nc.scalar.activation(v[:, :H], pv[:, :H],
                                     mybir.ActivationFunctionType.Copy)
                nc.vector.tensor_copy(v[:, H:], pv[:, H:])
                vs.append(v)

            # ---------- attnT for all groups (4 groups x 4 entities) --------
            attnTs = {}
            for grp in range(4):
                for sc in range(SCHUNK):
                    attnT = sb.tile([P, 4, HEADS * NE], F32R,
                                    tag=f"attnT{grp}_{sc}")
                    for k in range(4):
                        ent = grp * 4 + k
                        if k < 2:
                            nc.vector.tensor_scalar_mul(
                                attnT[:, k, :], e_sbs[sc][:],
                                masks_sb[:, sc, ent:ent + 1].bitcast(F32),
                            )
                        elif k == 2:
                            nc.scalar.activation(
                                attnT[:, k, :], e_sbs[sc][:],
                                mybir.ActivationFunctionType.Copy,
                                scale=masks_sb[:, sc, ent:ent + 1].bitcast(F32),
                            )
                        else:
                            nc.gpsimd.tensor_scalar_mul(
                                attnT[:, k, :], e_sbs[sc][:],
                                masks_sb[:, sc, ent:ent + 1].bitcast(F32),
                            )
                    attnTs[(grp, sc)] = attnT

            # ---------- PV -> normalize -> O, per 4-entity group ----------
            for grp in range(4):
                outT = sb.tile([P, NCHUNK, 4, NE], F32R, tag=f"outT{grp}")
                for h in range(HEADS):
                    for j in range(2):
                        dc = 2 * h + j
                        po = pbig.tile([P, 4 * NE], F32, tag="pb",
                                       name=f"pos_{grp}_{dc}")
                        for sc in range(SCHUNK):
                            nc.tensor.matmul(
                                po[:],
                                vs[sc][:, dc * P:(dc + 1) * P],
                                attnTs[(grp, sc)][:, :, h * NE:(h + 1) * NE],
                                start=(sc == 0), stop=(sc == SCHUNK - 1),
                            )
                        nc.vector.tensor_mul(
                            outT[:, dc, :, :], po[:],
                            srec_slice(grp, h),
                        )
                o_sb = ostage.tile([P, 2, NH], F32)
                for lp in range(2):
                    pair = grp * 2 + lp
                    pO = pbig.tile([P, NH], F32, tag="pb", name=f"pO{pair}")
                    for hc in range(NCHUNK):
                        nc.tensor.matmul(
                            pO[:], outT[:, hc, 2 * lp:2 * lp + 2, :],
                            wo_sb[:, hc, :],
                            start=(hc == 0), stop=(hc == NCHUNK - 1),
                        )
                    H2 = NH // 2
                    nc.vector.tensor_copy(o_sb[:, lp, :H2], pO[:, :H2])
                    nc.scalar.activation(o_sb[:, lp, H2:], pO[:, H2:],
                                         mybir.ActivationFunctionType.Copy)
                    if grp >= 2:
                        nc.sync.dma_start(
                            out_d[pair * P:(pair + 1) * P, :], o_sb[:, lp, :])
                if grp < 2:
                    base = grp * 2 * P
                    nc.sync.dma_start(
                        out_d[base:base + 2 * P, :].rearrange(
                            "(q p) d -> p q d", p=P),
                        o_sb[:])

    nc.compile()
    return nc


def _get_nc():
    if "nc" not in _CACHE:
        _CACHE["nc"] = _build()
    return _CACHE["nc"]


def _fast_run(nc, in_maps):
    """Repeat-call path: same PJRT execution as run_bass_kernel_spmd/
    bass2jax.run_bass_via_pjrt, but with the jitted shard_map cached so
    repeat kernel() calls skip retracing/relowering."""
    import jax
    import jax.numpy as jnp
    from jax.sharding import Mesh, PartitionSpec
    from jax.experimental.shard_map import shard_map
    import concourse.mybir as mybir_
    from concourse import bass2jax

    if "runner" not in _CACHE:
        bass2jax.install_neuronx_cc_hook()
        part_name = (nc.partition_id_tensor.name
                     if nc.partition_id_tensor else None)
        in_names, out_names, out_avals = [], [], []
        for alloc in nc.m.functions[0].allocations:
            if not isinstance(alloc, mybir_.MemoryLocationSet):
                continue
            name = alloc.memorylocations[0].name
            if alloc.kind == "ExternalInput":
                if name != part_name:
                    in_names.append(name)
            elif alloc.kind == "ExternalOutput":
                out_names.append(name)
                out_avals.append(jax.core.ShapedArray(
                    tuple(alloc.tensor_shape), mybir_.dt.np(alloc.dtype)))
        n_params = len(in_names)
        all_in_names = in_names + out_names
        if part_name is not None:
            all_in_names = all_in_names + [part_name]

        def _body(*args):
            operands = list(args)
            if part_name is not None:
                operands.append(bass2jax.partition_id_tensor())
            outs = bass2jax._bass_exec_p.bind(
                *operands,
                out_avals=tuple(out_avals),
                in_names=tuple(all_in_names),
                out_names=tuple(out_names),
                lowering_input_output_aliases=(),
                sim_require_finite=True,
                sim_require_nnan=True,
                nc=nc,
            )
            return tuple(outs)

        devices = jax.devices()[:NB]
        mesh = Mesh(np.asarray(devices), ("core",))
        n_outs = len(out_names)
        sharded = jax.jit(
            shard_map(_body, mesh=mesh,
                      in_specs=(PartitionSpec("core"),) * (n_params + n_outs),
                      out_specs=(PartitionSpec("core"),) * n_outs,
                      check_rep=False),
            donate_argnums=tuple(range(n_params, n_params + n_outs)),
            keep_unused=True,
        )
        _CACHE["runner"] = (sharded, in_names, out_names, out_avals)

    sharded, in_names, out_names, out_avals = _CACHE["runner"]
    concat_in = [
        np.concatenate([np.asarray(m[name]) for m in in_maps], axis=0)
        for name in in_names
    ]
    concat_zeros = [
        np.zeros((NB * av.shape[0], *av.shape[1:]), av.dtype)
        for av in out_avals
    ]
    out_arrs = sharded(*concat_in, *concat_zeros)
    return [
        {name: np.asarray(out_arrs[i]).reshape(NB, *out_avals[i].shape)[c]
         for i, name in enumerate(out_names)}
        for c in range(NB)
    ]


def kernel(tokens_embed, entities, events_embed, entity_num, entity_masks,
           select_event, Wq, Wk, Wv, bq, bk, bv, Wo, bo):
    tokens_embed = np.asarray(tokens_embed, dtype=np.float32)
    entities = np.asarray(entities)
    events_embed = np.asarray(events_embed, dtype=np.float32)
    entity_masks = np.asarray(entity_masks)
    select_event = np.asarray(select_event)
    Wq = np.asarray(Wq, dtype=np.float32)
    Wk = np.asarray(Wk, dtype=np.float32)
    Wv = np.asarray(Wv, dtype=np.float32)
    Wo = np.asarray(Wo, dtype=np.float32)
    bq = np.asarray(bq, dtype=np.float32)
    bk = np.asarray(bk, dtype=np.float32)
    bv = np.asarray(bv, dtype=np.float32)
    bo = np.asarray(bo, dtype=np.float32)

    nc = _get_nc()

    q_s = (events_embed @ Wq.T + bq) * SCALE          # [NE, NH]
    # fold the K projection into the query side (bk cancels in softmax):
    # wtil[hid, (h,e)] = sum_dout_in_head Wk[dout, hid] * q_s[e, dout]
    wtil = np.empty((NH, HEADS * NE), dtype=np.float32)
    for h in range(HEADS):
        hs = slice(h * DH, (h + 1) * DH)
        wtil[:, h * NE:(h + 1) * NE] = (q_s[:, hs] @ Wk[hs, :]).T
    wtil_pc = np.ascontiguousarray(
        wtil.reshape(NCHUNK, P, HEADS * NE).transpose(1, 0, 2))
    # attn rows sum to 1, so the bv term of out contributes bv @ Wo.T to O;
    # the whole output bias is applied host-side after the gather.
    bo2 = (bo + bv @ Wo.T).astype(np.float32)
    smalls = np.zeros((P, 64), dtype=np.float32)
    shared = {
        "wtil": wtil_pc,
        "WvT": np.ascontiguousarray(Wv.T),
        "WoT": np.ascontiguousarray(Wo.T),
    }
    in_maps = []
    for c in range(NB):
        # masksT[p, sc, ent] = entities[c, ent, sc*128 + p]
        m = entities[c].astype(np.float32)            # [EN, SL]
        mT = m.reshape(EN, SCHUNK, P).transpose(2, 1, 0).reshape(P, -1)
        sm = smalls.copy()
        sm[:, 0:64] = mT
        in_maps.append({
            "toksT": np.ascontiguousarray(tokens_embed[c].T),
            "smalls": sm,
            **shared,
        })

    if "ran_once" not in _CACHE:
        res = run_bass_kernel_spmd(nc, in_maps, core_ids=list(range(NB)))
        results = res.results
        _CACHE["ran_once"] = True
    else:
        results = _fast_run(nc, in_maps)
    full = np.concatenate([results[c]["out"] for c in range(NB)], axis=0)
    full += bo2[None, :]
    # full[(b*EN + ent)*NE + e] = attention output for group (b, ent), event e

    # ragged selection (mirrors the reference indexing; identity for the
    # all-ones masks produced by setup_inputs)
    assert int(entity_num) == EN
    entity_index = np.flatnonzero(entity_masks.reshape(-1))
    pair_sel = (select_event[:, None, :] & entity_masks[:, :, None])
    pair_sel = pair_sel.reshape(-1, NE)[entity_index].reshape(-1)
    event_entity_index = np.flatnonzero(pair_sel)

    sel_rows = (entity_index[:, None] * NE + np.arange(NE)[None, :]).reshape(-1)
    return full[sel_rows][event_entity_index]

